# revision 3
# baseline (speedup 1.0000x reference)
"""CavemanGPT single-head attention on 8 Trainium2 NeuronCores.

Math (reference, mask is unused there):
    Q = emb @ W_q^T ; K = emb @ W_k^T ; V = emb @ W_v^T        (per batch b)
    out = softmax(K @ Q^T / sqrt(H), axis=-1) @ V

Key algebraic restructure: K @ Q^T = emb @ (W_k^T W_q) @ emb^T, so with
G := W_k^T @ W_q  ([E, E], batch independent) the per-core work drops from
~52 GFLOP to ~24 GFLOP and the giant [S, H] Q/K intermediates vanish:
    AT := G^T @ emb_i^T            ([E, SI])
    scores = AT^T @ emb^T          ([SI, S])
    out = softmax(scores/sqrt(H)) @ V

Sharding: 8 cores = 4 batches x 2 halves of the i (output-row) dimension.
Each core receives its batch's emb with its own i-half permuted to the front
(softmax over j is permutation invariant), computes G redundantly, and
produces out[i-half].

Precision: the scores chain needs fp32-grade accuracy (softmax here is a
near-argmax; see top-2 gap analysis), but plain fp32 matmuls run at 1/4 rate
on the PE. float32r (fp32 with 11-bit mantissa) streams at full rate, so the
G and AT stages use a hi/lo split (x = xh + xl, both fp32r): 3 full-rate
matmuls give fp32-grade products. The scores stage itself runs plain fp32
(operands AT fp32 + emb fp32 cached in SBUF). V and the attn@V stage are
post-softmax (error passes through linearly) and use single fp32r.
"""

import math

import numpy as np

import concourse.bass as bass
import concourse.mybir as mybir
import concourse.tile as tile
from concourse import bacc
from concourse.bass_utils import run_bass_kernel_spmd
from concourse.masks import make_identity

dt = mybir.dt
P = 128


def _round12(x):
    """Round fp32 array to fp32r (1+8+11 bits) with round-to-nearest."""
    u = x.view(np.uint32)
    kept = (u >> 12) & np.uint32(1)
    u2 = (u.astype(np.uint64) + 0x7FF + kept.astype(np.uint64)) & np.uint64(0xFFFFF000)
    return u2.astype(np.uint32).view(np.float32)


def _split12(x):
    x = np.ascontiguousarray(x, dtype=np.float32)
    hi = _round12(x)
    lo = _round12((x - hi).astype(np.float32))
    return hi, lo


def build_nc(S, E, H, O):
    """Build + compile the per-core Bass program."""
    SI = S // 2          # i rows per core
    EB = E // P          # 128-chunks of the embedding dim
    HB = H // P
    JB = S // P
    IB = SI // P
    GW = min(512, E)     # PSUM bank width for G along e'
    NGB = E // GW
    IW = min(512, SI)    # AT moving width along i
    NIH = SI // IW
    JW = min(512, S)     # scores moving width along j
    NJW = S // JW
    OW = min(512, O)
    NOW = O // OW
    SCALE = 1.0 / math.sqrt(H)

    f32, f32r = dt.float32, dt.float32r

    nc = bacc.Bacc("TRN2", target_bir_lowering=False, debug=False)
    wk_h = nc.dram_tensor("wk_h", [H, E], f32r, kind="ExternalInput").ap()
    wk_l = nc.dram_tensor("wk_l", [H, E], f32r, kind="ExternalInput").ap()
    wq_h = nc.dram_tensor("wq_h", [H, E], f32r, kind="ExternalInput").ap()
    wq_l = nc.dram_tensor("wq_l", [H, E], f32r, kind="ExternalInput").ap()
    et_h = nc.dram_tensor("et_h", [E, S], f32r, kind="ExternalInput").ap()
    et_l = nc.dram_tensor("et_l", [E, S], f32r, kind="ExternalInput").ap()
    et_f = nc.dram_tensor("et_f", [E, S], f32, kind="ExternalInput").ap()
    wvt = nc.dram_tensor("wvt", [E, O], f32r, kind="ExternalInput").ap()
    out = nc.dram_tensor("out", [SI, O], f32, kind="ExternalOutput").ap()

    with tile.TileContext(nc) as tc:
        with (
            tc.tile_pool(name="misc", bufs=1) as misc,
            tc.tile_pool(name="p_at", bufs=1) as p_at,
        ):
            ident = misc.tile([P, P], f32)
            make_identity(nc, ident[:])
            at = p_at.tile([P, EB, SI], f32)  # AT: [e' part, e' chunk, i]

            with tc.tile_pool(name="p_g", bufs=1) as p_g:
                gh = p_g.tile([P, EB, E], f32r)  # G: [e part, e chunk, e']
                gl = p_g.tile([P, EB, E], f32r)

                # ---- G = W_k^T @ W_q (hi/lo split x3) ----
                n_gtiles = EB * NGB
                sets = max(1, (n_gtiles + 7) // 8)
                assert EB % sets == 0
                EBS = EB // sets
                with (
                    tc.tile_pool(name="p_gs", bufs=2) as p_gs,
                    tc.tile_pool(name="ps_g", bufs=8, space="PSUM") as ps_g,
                ):
                    for s in range(sets):
                        pt_g = [
                            [
                                ps_g.tile([P, GW], f32, tag="gps", name=f"gps_{s}_{i}_{nb}")
                                for nb in range(NGB)
                            ]
                            for i in range(EBS)
                        ]
                        for hc in range(HB):
                            hs = slice(hc * P, (hc + 1) * P)
                            es = slice(s * EBS * P, (s + 1) * EBS * P)
                            kh = p_gs.tile([P, EBS * P], f32r, tag="kh")
                            nc.sync.dma_start(kh[:], wk_h[hs, es])
                            kl = p_gs.tile([P, EBS * P], f32r, tag="kl")
                            nc.sync.dma_start(kl[:], wk_l[hs, es])
                            qh = p_gs.tile([P, E], f32r, tag="qh")
                            nc.sync.dma_start(qh[:], wq_h[hs, :])
                            ql = p_gs.tile([P, E], f32r, tag="ql")
                            nc.sync.dma_start(ql[:], wq_l[hs, :])
                            first, last = hc == 0, hc == HB - 1
                            for i in range(EBS):
                                ksl = slice(i * P, (i + 1) * P)
                                for nb in range(NGB):
                                    nsl = slice(nb * GW, (nb + 1) * GW)
                                    pt = pt_g[i][nb]
                                    nc.tensor.matmul(
                                        pt[:], kh[:, ksl], qh[:, nsl],
                                        start=first, stop=False,
                                    )
                                    nc.tensor.matmul(
                                        pt[:], kh[:, ksl], ql[:, nsl],
                                        start=False, stop=False,
                                    )
                                    nc.tensor.matmul(
                                        pt[:], kl[:, ksl], qh[:, nsl],
                                        start=False, stop=last,
                                    )
                        for i in range(EBS):
                            eb = s * EBS + i
                            for nb in range(NGB):
                                nsl = slice(nb * GW, (nb + 1) * GW)
                                pt = pt_g[i][nb]
                                nc.vector.tensor_copy(gh[:, eb, nsl], pt[:])
                                nc.vector.tensor_tensor(
                                    gl[:, eb, nsl], pt[:], gh[:, eb, nsl],
                                    mybir.AluOpType.subtract,
                                )

                # ---- AT[e', i] = sum_e G[e, e'] * embT[e, i] (split x3) ----
                with (
                    tc.tile_pool(name="p_ec", bufs=1) as p_ec,
                    tc.tile_pool(name="ps_a", bufs=4, space="PSUM") as ps_a,
                ):
                    for ih in range(NIH):
                        isl = slice(ih * IW, (ih + 1) * IW)
                        ehc = p_ec.tile([P, EB, IW], f32r, tag="ehc")
                        nc.sync.dma_start(
                            ehc[:],
                            et_h[:, isl].rearrange("(eo p) i -> p eo i", p=P),
                        )
                        elc = p_ec.tile([P, EB, IW], f32r, tag="elc")
                        nc.sync.dma_start(
                            elc[:],
                            et_l[:, isl].rearrange("(eo p) i -> p eo i", p=P),
                        )
                        for epb in range(EB):
                            psl = slice(epb * P, (epb + 1) * P)
                            pt = ps_a.tile([P, IW], f32, tag="aps", name=f"aps_{ih}_{epb}")
                            for eb in range(EB):
                                first, last = eb == 0, eb == EB - 1
                                nc.tensor.matmul(
                                    pt[:], gh[:, eb, psl], ehc[:, eb, :],
                                    start=first, stop=False,
                                )
                                nc.tensor.matmul(
                                    pt[:], gh[:, eb, psl], elc[:, eb, :],
                                    start=False, stop=False,
                                )
                                nc.tensor.matmul(
                                    pt[:], gl[:, eb, psl], ehc[:, eb, :],
                                    start=False, stop=last,
                                )
                            nc.vector.tensor_copy(at[:, epb, isl], pt[:])

            # ---- V[j, o] = sum_e embT[e, j] * WvT[e, o] (single fp32r) ----
            with tc.tile_pool(name="p_v", bufs=1) as p_v:
                v = p_v.tile([P, JB, O], f32r)  # [j part, j chunk, o]
                with (
                    tc.tile_pool(name="p_vw", bufs=1) as p_vw,
                    tc.tile_pool(name="p_vs", bufs=3) as p_vs,
                    tc.tile_pool(name="ps_v", bufs=4, space="PSUM") as ps_v,
                ):
                    wvc = p_vw.tile([P, EB, O], f32r)
                    nc.sync.dma_start(
                        wvc[:], wvt.rearrange("(eo p) o -> p eo o", p=P)
                    )
                    for jb in range(JB):
                        jsl = slice(jb * P, (jb + 1) * P)
                        est = p_vs.tile([P, EB, P], f32r, tag="est")
                        nc.sync.dma_start(
                            est[:],
                            et_h[:, jsl].rearrange("(eo p) j -> p eo j", p=P),
                        )
                        for ob in range(NOW):
                            osl = slice(ob * OW, (ob + 1) * OW)
                            pv = ps_v.tile([P, OW], f32, tag="vps", name=f"vps_{jb}_{ob}")
                            for eb in range(EB):
                                nc.tensor.matmul(
                                    pv[:], est[:, eb, :], wvc[:, eb, osl],
                                    start=(eb == 0), stop=(eb == EB - 1),
                                )
                            nc.vector.tensor_copy(v[:, jb, osl], pv[:])

                # ---- scores + softmax + out, fused per 128-row i block ----
                with (
                    tc.tile_pool(name="p_et", bufs=1) as p_et,
                    tc.tile_pool(name="p_sw", bufs=2) as p_sw,
                    tc.tile_pool(name="p_sw1", bufs=1) as p_sw1,
                    tc.tile_pool(name="ps_s", bufs=4, space="PSUM") as ps_s,
                    tc.tile_pool(name="ps_t", bufs=2, space="PSUM") as ps_t,
                    tc.tile_pool(name="ps_o", bufs=2, space="PSUM") as ps_o,
                ):
                    etf = p_et.tile([P, EB, S], f32)
                    nc.sync.dma_start(
                        etf[:], et_f.rearrange("(eo p) j -> p eo j", p=P)
                    )
                    for ib in range(IB):
                        ibs = slice(ib * P, (ib + 1) * P)
                        pt_s = [
                            ps_s.tile([P, JW], f32, tag="sps", name=f"sps_{ib}_{w}")
                            for w in range(NJW)
                        ]
                        for epb in range(EB):
                            for w in range(NJW):
                                nc.tensor.matmul(
                                    pt_s[w][:],
                                    at[:, epb, ibs],
                                    etf[:, epb, w * JW : (w + 1) * JW],
                                    start=(epb == 0), stop=(epb == EB - 1),
                                )
                        sc = p_sw.tile([P, S], f32, tag="sc")
                        for w in range(NJW):
                            nc.vector.tensor_copy(
                                sc[:, w * JW : (w + 1) * JW], pt_s[w][:]
                            )
                        nmx = p_sw.tile([P, 1], f32, tag="nmx")
                        nc.vector.reduce_max(
                            nmx[:], sc[:], axis=mybir.AxisListType.X, negate=True
                        )
                        nmx2 = p_sw.tile([P, 1], f32, tag="nmx2")
                        nc.vector.tensor_scalar_mul(nmx2[:], nmx[:], SCALE)
                        nc.scalar.activation(
                            sc[:], sc[:], mybir.ActivationFunctionType.Exp,
                            bias=nmx2[:], scale=SCALE,
                        )
                        sm = p_sw.tile([P, 1], f32, tag="sm")
                        nc.vector.reduce_sum(sm[:], sc[:], axis=mybir.AxisListType.X)
                        rs = p_sw.tile([P, 1], f32, tag="rs")
                        nc.vector.reciprocal(rs[:], sm[:])
                        nc.vector.tensor_scalar_mul(sc[:], sc[:], rs[:])
                        attnT = p_sw1.tile([P, JB, P], f32r, tag="attnT")
                        for jb in range(JB):
                            tp = ps_t.tile([P, P], f32, tag="tps", name=f"tps_{ib}_{jb}")
                            nc.tensor.transpose(
                                tp[:], sc[:, jb * P : (jb + 1) * P], ident[:]
                            )
                            nc.vector.tensor_copy(attnT[:, jb, :], tp[:])
                        pt_o = [
                            ps_o.tile([P, OW], f32, tag="ops", name=f"ops_{ib}_{ob}")
                            for ob in range(NOW)
                        ]
                        for jb in range(JB):
                            for ob in range(NOW):
                                nc.tensor.matmul(
                                    pt_o[ob][:],
                                    attnT[:, jb, :],
                                    v[:, jb, ob * OW : (ob + 1) * OW],
                                    start=(jb == 0), stop=(jb == JB - 1),
                                )
                        outt = p_sw1.tile([P, O], f32, tag="outt")
                        for ob in range(NOW):
                            nc.vector.tensor_copy(
                                outt[:, ob * OW : (ob + 1) * OW], pt_o[ob][:]
                            )
                        nc.sync.dma_start(out[ibs, :], outt[:])

    nc.compile()
    return nc


def host_prepare(token_emb, W_q, W_k, W_v):
    """Shard + format-convert inputs for the 8 cores."""
    token_emb = np.ascontiguousarray(token_emb, np.float32)
    B, S, E = token_emb.shape
    SI = S // 2
    wk_h, wk_l = _split12(W_k)
    wq_h, wq_l = _split12(W_q)
    wvt = _round12(np.ascontiguousarray(np.asarray(W_v, np.float32).T))

    in_maps = []
    for c in range(2 * B):
        b, half = divmod(c, 2)
        e = token_emb[b]
        perm = np.concatenate(
            [e[half * SI : (half + 1) * SI], e[(1 - half) * SI : (2 - half) * SI]], axis=0
        )
        et_f = np.ascontiguousarray(perm.T)  # [E, S]
        et_h, et_l = _split12(et_f)
        in_maps.append(
            {
                "wk_h": wk_h, "wk_l": wk_l, "wq_h": wq_h, "wq_l": wq_l,
                "et_h": et_h, "et_l": et_l, "et_f": et_f, "wvt": wvt,
            }
        )
    return in_maps


_NC_CACHE = {}


def _get_nc(S, E, H, O):
    key = (S, E, H, O)
    if key not in _NC_CACHE:
        _NC_CACHE[key] = build_nc(S, E, H, O)
    return _NC_CACHE[key]


def kernel(token_emb, W_q, W_k, W_v, mask=None, _trace=False, _tmpdir=None):
    token_emb = np.asarray(token_emb, np.float32)
    W_q = np.asarray(W_q, np.float32)
    W_k = np.asarray(W_k, np.float32)
    W_v = np.asarray(W_v, np.float32)
    B, S, E = token_emb.shape
    H = W_q.shape[0]
    O = W_v.shape[0]
    SI = S // 2

    nc = _get_nc(S, E, H, O)
    in_maps = host_prepare(token_emb, W_q, W_k, W_v)
    res = run_bass_kernel_spmd(
        nc, in_maps, core_ids=list(range(2 * B)), trace=_trace, tmpdir=_tmpdir
    )

    out = np.empty((B, S, O), np.float32)
    for c in range(2 * B):
        b, half = divmod(c, 2)
        out[b, half * SI : (half + 1) * SI] = res.results[c]["out"]
    if _trace:
        kernel._last_results = res
    return out


# revision 4
# speedup vs baseline: 1.0161x; 1.0161x over previous
"""CavemanGPT single-head attention on 8 Trainium2 NeuronCores.

Math (reference, mask is unused there):
    Q = emb @ W_q^T ; K = emb @ W_k^T ; V = emb @ W_v^T        (per batch b)
    out = softmax(K @ Q^T / sqrt(H), axis=-1) @ V

Key algebraic restructure: K @ Q^T = emb @ (W_k^T W_q) @ emb^T, so with
G := W_k^T @ W_q  ([E, E], batch independent) the per-core work drops from
~52 GFLOP to ~24 GFLOP and the giant [S, H] Q/K intermediates vanish:
    AT := G^T @ emb_i^T            ([E, SI])
    scores = AT^T @ emb^T          ([SI, S])
    out = softmax(scores/sqrt(H)) @ V

Sharding: 8 cores = 4 batches x 2 halves of the i (output-row) dimension.
Each core receives its batch's emb with its own i-half permuted to the front
(softmax over j is permutation invariant), computes G redundantly, and
produces out[i-half].

Precision: the scores chain needs fp32-grade accuracy (softmax here is a
near-argmax; see top-2 gap analysis), but plain fp32 matmuls run at 1/4 rate
on the PE. float32r (fp32 with 11-bit mantissa) streams at full rate, so the
G and AT stages use a hi/lo split (x = xh + xl, both fp32r): 3 full-rate
matmuls give fp32-grade products. The scores stage itself runs plain fp32
(operands AT fp32 + emb fp32 cached in SBUF). V and the attn@V stage are
post-softmax (error passes through linearly) and use single fp32r.
"""

import math

import numpy as np

import concourse.bass as bass
import concourse.bass_utils as _bu
import concourse.mybir as mybir
import concourse.tile as tile
from concourse import bacc
from concourse.bass_utils import run_bass_kernel_spmd
from concourse.masks import make_identity

# LDWEIGHTS dedup: consecutive matmuls sharing a stationary operand skip the
# reload. Verified to produce bit-identical output on this kernel.
if not getattr(_bu, "_ldw_opt_patched", False):
    _orig_walrus_args = _bu.get_walrus_args

    def _walrus_args_ldw(arch, tmpdir, *, dve_root=None):
        args = _orig_walrus_args(arch, tmpdir, dve_root=dve_root)
        return [a.replace("--enable-ldw-opt=false", "--enable-ldw-opt=true") for a in args]

    _bu.get_walrus_args = _walrus_args_ldw
    _bu._ldw_opt_patched = True

dt = mybir.dt
P = 128


def _round12(x):
    """Round fp32 array to fp32r (1+8+11 bits) with round-to-nearest."""
    u = x.view(np.uint32)
    kept = (u >> 12) & np.uint32(1)
    u2 = (u.astype(np.uint64) + 0x7FF + kept.astype(np.uint64)) & np.uint64(0xFFFFF000)
    return u2.astype(np.uint32).view(np.float32)


def _split12(x):
    x = np.ascontiguousarray(x, dtype=np.float32)
    hi = _round12(x)
    lo = _round12((x - hi).astype(np.float32))
    return hi, lo


def build_nc(S, E, H, O):
    """Build + compile the per-core Bass program."""
    SI = S // 2          # i rows per core
    EB = E // P          # 128-chunks of the embedding dim
    HB = H // P
    JB = S // P
    IB = SI // P
    GW = min(512, E)     # PSUM bank width for G along e'
    NGB = E // GW
    IW = min(512, SI)    # AT moving width along i
    NIH = SI // IW
    JW = min(512, S)     # scores moving width along j
    NJW = S // JW
    OW = min(512, O)
    NOW = O // OW
    SCALE = 1.0 / math.sqrt(H)

    f32, f32r = dt.float32, dt.float32r

    nc = bacc.Bacc("TRN2", target_bir_lowering=False, debug=False)
    wk_h = nc.dram_tensor("wk_h", [H, E], f32r, kind="ExternalInput").ap()
    wk_l = nc.dram_tensor("wk_l", [H, E], f32r, kind="ExternalInput").ap()
    wq_h = nc.dram_tensor("wq_h", [H, E], f32r, kind="ExternalInput").ap()
    wq_l = nc.dram_tensor("wq_l", [H, E], f32r, kind="ExternalInput").ap()
    et_h = nc.dram_tensor("et_h", [E, S], f32r, kind="ExternalInput").ap()
    et_l = nc.dram_tensor("et_l", [E, S], f32r, kind="ExternalInput").ap()
    et_f = nc.dram_tensor("et_f", [E, S], f32, kind="ExternalInput").ap()
    wvt = nc.dram_tensor("wvt", [E, O], f32r, kind="ExternalInput").ap()
    out = nc.dram_tensor("out", [SI, O], f32, kind="ExternalOutput").ap()

    with tile.TileContext(nc) as tc:
        with (
            tc.tile_pool(name="misc", bufs=1) as misc,
            tc.tile_pool(name="p_at", bufs=1) as p_at,
        ):
            ident = misc.tile([P, P], f32)
            make_identity(nc, ident[:])
            at = p_at.tile([P, EB, SI], f32)  # AT: [e' part, e' chunk, i]

            with tc.tile_pool(name="p_g", bufs=1) as p_g:
                gh = p_g.tile([P, EB, E], f32r)  # G: [e part, e chunk, e']
                gl = p_g.tile([P, EB, E], f32r)

                # ---- G = W_k^T @ W_q (hi/lo split x3) ----
                n_gtiles = EB * NGB
                sets = max(1, (n_gtiles + 7) // 8)
                assert EB % sets == 0
                EBS = EB // sets
                with (
                    tc.tile_pool(name="p_gs", bufs=2) as p_gs,
                    tc.tile_pool(name="ps_g", bufs=8, space="PSUM") as ps_g,
                ):
                    for s in range(sets):
                        pt_g = [
                            [
                                ps_g.tile([P, GW], f32, tag="gps", name=f"gps_{s}_{i}_{nb}")
                                for nb in range(NGB)
                            ]
                            for i in range(EBS)
                        ]
                        for hc in range(HB):
                            hs = slice(hc * P, (hc + 1) * P)
                            es = slice(s * EBS * P, (s + 1) * EBS * P)
                            kh = p_gs.tile([P, EBS * P], f32r, tag="kh")
                            nc.sync.dma_start(kh[:], wk_h[hs, es])
                            kl = p_gs.tile([P, EBS * P], f32r, tag="kl")
                            nc.sync.dma_start(kl[:], wk_l[hs, es])
                            qh = p_gs.tile([P, E], f32r, tag="qh")
                            nc.sync.dma_start(qh[:], wq_h[hs, :])
                            ql = p_gs.tile([P, E], f32r, tag="ql")
                            nc.sync.dma_start(ql[:], wq_l[hs, :])
                            first, last = hc == 0, hc == HB - 1
                            for i in range(EBS):
                                ksl = slice(i * P, (i + 1) * P)
                                for nb in range(NGB):
                                    nsl = slice(nb * GW, (nb + 1) * GW)
                                    pt = pt_g[i][nb]
                                    nc.tensor.matmul(
                                        pt[:], kh[:, ksl], qh[:, nsl],
                                        start=first, stop=False,
                                    )
                                    nc.tensor.matmul(
                                        pt[:], kh[:, ksl], ql[:, nsl],
                                        start=False, stop=False,
                                    )
                                    nc.tensor.matmul(
                                        pt[:], kl[:, ksl], qh[:, nsl],
                                        start=False, stop=last,
                                    )
                        for i in range(EBS):
                            eb = s * EBS + i
                            for nb in range(NGB):
                                nsl = slice(nb * GW, (nb + 1) * GW)
                                pt = pt_g[i][nb]
                                nc.vector.tensor_copy(gh[:, eb, nsl], pt[:])
                                nc.vector.tensor_tensor(
                                    gl[:, eb, nsl], pt[:], gh[:, eb, nsl],
                                    mybir.AluOpType.subtract,
                                )

                # ---- AT[e', i] = sum_e G[e, e'] * embT[e, i] (split x3) ----
                with (
                    tc.tile_pool(name="p_ec", bufs=1) as p_ec,
                    tc.tile_pool(name="ps_a", bufs=4, space="PSUM") as ps_a,
                ):
                    for ih in range(NIH):
                        isl = slice(ih * IW, (ih + 1) * IW)
                        ehc = p_ec.tile([P, EB, IW], f32r, tag="ehc")
                        nc.sync.dma_start(
                            ehc[:],
                            et_h[:, isl].rearrange("(eo p) i -> p eo i", p=P),
                        )
                        elc = p_ec.tile([P, EB, IW], f32r, tag="elc")
                        nc.sync.dma_start(
                            elc[:],
                            et_l[:, isl].rearrange("(eo p) i -> p eo i", p=P),
                        )
                        for epb in range(EB):
                            psl = slice(epb * P, (epb + 1) * P)
                            pt = ps_a.tile([P, IW], f32, tag="aps", name=f"aps_{ih}_{epb}")
                            for eb in range(EB):
                                first, last = eb == 0, eb == EB - 1
                                nc.tensor.matmul(
                                    pt[:], gh[:, eb, psl], ehc[:, eb, :],
                                    start=first, stop=False,
                                )
                                nc.tensor.matmul(
                                    pt[:], gh[:, eb, psl], elc[:, eb, :],
                                    start=False, stop=False,
                                )
                                nc.tensor.matmul(
                                    pt[:], gl[:, eb, psl], ehc[:, eb, :],
                                    start=False, stop=last,
                                )
                            nc.vector.tensor_copy(at[:, epb, isl], pt[:])

            # ---- V[j, o] = sum_e embT[e, j] * WvT[e, o] (single fp32r) ----
            with tc.tile_pool(name="p_v", bufs=1) as p_v:
                v = p_v.tile([P, JB, O], f32r)  # [j part, j chunk, o]
                with (
                    tc.tile_pool(name="p_vw", bufs=1) as p_vw,
                    tc.tile_pool(name="p_vs", bufs=3) as p_vs,
                    tc.tile_pool(name="ps_v", bufs=4, space="PSUM") as ps_v,
                ):
                    wvc = p_vw.tile([P, EB, O], f32r)
                    nc.sync.dma_start(
                        wvc[:], wvt.rearrange("(eo p) o -> p eo o", p=P)
                    )
                    for jb in range(JB):
                        jsl = slice(jb * P, (jb + 1) * P)
                        est = p_vs.tile([P, EB, P], f32r, tag="est")
                        nc.sync.dma_start(
                            est[:],
                            et_h[:, jsl].rearrange("(eo p) j -> p eo j", p=P),
                        )
                        for ob in range(NOW):
                            osl = slice(ob * OW, (ob + 1) * OW)
                            pv = ps_v.tile([P, OW], f32, tag="vps", name=f"vps_{jb}_{ob}")
                            for eb in range(EB):
                                nc.tensor.matmul(
                                    pv[:], est[:, eb, :], wvc[:, eb, osl],
                                    start=(eb == 0), stop=(eb == EB - 1),
                                )
                            nc.vector.tensor_copy(v[:, jb, osl], pv[:])

                # ---- scores + softmax + out, fused per 128-row i block ----
                with (
                    tc.tile_pool(name="p_et", bufs=1) as p_et,
                    tc.tile_pool(name="p_sw", bufs=2) as p_sw,
                    tc.tile_pool(name="p_sw1", bufs=1) as p_sw1,
                    tc.tile_pool(name="ps_s", bufs=4, space="PSUM") as ps_s,
                    tc.tile_pool(name="ps_t", bufs=2, space="PSUM") as ps_t,
                    tc.tile_pool(name="ps_o", bufs=2, space="PSUM") as ps_o,
                ):
                    etf = p_et.tile([P, EB, S], f32)
                    nc.sync.dma_start(
                        etf[:], et_f.rearrange("(eo p) j -> p eo j", p=P)
                    )
                    for ib in range(IB):
                        ibs = slice(ib * P, (ib + 1) * P)
                        pt_s = [
                            ps_s.tile([P, JW], f32, tag="sps", name=f"sps_{ib}_{w}")
                            for w in range(NJW)
                        ]
                        for epb in range(EB):
                            for w in range(NJW):
                                nc.tensor.matmul(
                                    pt_s[w][:],
                                    at[:, epb, ibs],
                                    etf[:, epb, w * JW : (w + 1) * JW],
                                    start=(epb == 0), stop=(epb == EB - 1),
                                )
                        sc = p_sw.tile([P, S], f32, tag="sc")
                        for w in range(NJW):
                            nc.vector.tensor_copy(
                                sc[:, w * JW : (w + 1) * JW], pt_s[w][:]
                            )
                        nmx = p_sw.tile([P, 1], f32, tag="nmx")
                        nc.vector.reduce_max(
                            nmx[:], sc[:], axis=mybir.AxisListType.X, negate=True
                        )
                        nmx2 = p_sw.tile([P, 1], f32, tag="nmx2")
                        nc.vector.tensor_scalar_mul(nmx2[:], nmx[:], SCALE)
                        nc.scalar.activation(
                            sc[:], sc[:], mybir.ActivationFunctionType.Exp,
                            bias=nmx2[:], scale=SCALE,
                        )
                        sm = p_sw.tile([P, 1], f32, tag="sm")
                        nc.vector.reduce_sum(sm[:], sc[:], axis=mybir.AxisListType.X)
                        rs = p_sw.tile([P, 1], f32, tag="rs")
                        nc.vector.reciprocal(rs[:], sm[:])
                        nc.vector.tensor_scalar_mul(sc[:], sc[:], rs[:])
                        attnT = p_sw1.tile([P, JB, P], f32r, tag="attnT")
                        for jb in range(JB):
                            tp = ps_t.tile([P, P], f32, tag="tps", name=f"tps_{ib}_{jb}")
                            nc.tensor.transpose(
                                tp[:], sc[:, jb * P : (jb + 1) * P], ident[:]
                            )
                            nc.vector.tensor_copy(attnT[:, jb, :], tp[:])
                        pt_o = [
                            ps_o.tile([P, OW], f32, tag="ops", name=f"ops_{ib}_{ob}")
                            for ob in range(NOW)
                        ]
                        for jb in range(JB):
                            for ob in range(NOW):
                                nc.tensor.matmul(
                                    pt_o[ob][:],
                                    attnT[:, jb, :],
                                    v[:, jb, ob * OW : (ob + 1) * OW],
                                    start=(jb == 0), stop=(jb == JB - 1),
                                )
                        outt = p_sw1.tile([P, O], f32, tag="outt")
                        for ob in range(NOW):
                            nc.vector.tensor_copy(
                                outt[:, ob * OW : (ob + 1) * OW], pt_o[ob][:]
                            )
                        nc.sync.dma_start(out[ibs, :], outt[:])

    nc.compile()
    return nc


def host_prepare(token_emb, W_q, W_k, W_v):
    """Shard + format-convert inputs for the 8 cores."""
    token_emb = np.ascontiguousarray(token_emb, np.float32)
    B, S, E = token_emb.shape
    SI = S // 2
    wk_h, wk_l = _split12(W_k)
    wq_h, wq_l = _split12(W_q)
    wvt = _round12(np.ascontiguousarray(np.asarray(W_v, np.float32).T))

    in_maps = []
    for c in range(2 * B):
        b, half = divmod(c, 2)
        e = token_emb[b]
        perm = np.concatenate(
            [e[half * SI : (half + 1) * SI], e[(1 - half) * SI : (2 - half) * SI]], axis=0
        )
        et_f = np.ascontiguousarray(perm.T)  # [E, S]
        et_h, et_l = _split12(et_f)
        in_maps.append(
            {
                "wk_h": wk_h, "wk_l": wk_l, "wq_h": wq_h, "wq_l": wq_l,
                "et_h": et_h, "et_l": et_l, "et_f": et_f, "wvt": wvt,
            }
        )
    return in_maps


_NC_CACHE = {}


def _get_nc(S, E, H, O):
    key = (S, E, H, O)
    if key not in _NC_CACHE:
        _NC_CACHE[key] = build_nc(S, E, H, O)
    return _NC_CACHE[key]


def kernel(token_emb, W_q, W_k, W_v, mask=None, _trace=False, _tmpdir=None):
    token_emb = np.asarray(token_emb, np.float32)
    W_q = np.asarray(W_q, np.float32)
    W_k = np.asarray(W_k, np.float32)
    W_v = np.asarray(W_v, np.float32)
    B, S, E = token_emb.shape
    H = W_q.shape[0]
    O = W_v.shape[0]
    SI = S // 2

    nc = _get_nc(S, E, H, O)
    in_maps = host_prepare(token_emb, W_q, W_k, W_v)
    res = run_bass_kernel_spmd(
        nc, in_maps, core_ids=list(range(2 * B)), trace=_trace, tmpdir=_tmpdir
    )

    out = np.empty((B, S, O), np.float32)
    for c in range(2 * B):
        b, half = divmod(c, 2)
        out[b, half * SI : (half + 1) * SI] = res.results[c]["out"]
    if _trace:
        kernel._last_results = res
    return out


# revision 9
# speedup vs baseline: 1.6723x; 1.6459x over previous
"""CavemanGPT single-head attention on 8 Trainium2 NeuronCores.

Math (reference; its mask input is unused there):
    Q = emb @ W_q^T ; K = emb @ W_k^T ; V = emb @ W_v^T        (per batch b)
    out = softmax(K @ Q^T / sqrt(H), axis=-1) @ V

Key algebraic restructure: K @ Q^T = emb @ (W_k^T W_q) @ emb^T, so with
G := W_k^T @ W_q  ([E, E], batch independent) the per-core work drops from
~52 GFLOP to ~16 GFLOP and the giant [S, H] Q/K intermediates vanish:
    AT := (G^T @ emb_i^T) / 64     ([E, SI])
    scores = AT^T @ emb^T          ([SI, S], = true scores / 2)
    out = softmax(...) @ V

Two launches:
  1. G-launch: G = W_k^T @ W_q sharded over 8 cores (2 e'-halves x 4
     h-quarters); host sums the h-partials (in fp64).
  2. Main launch: 8 cores = 4 batches x 2 halves of the i (output-row)
     dimension. Each core receives its batch's emb with its own i-half
     permuted to the front (softmax over j is permutation invariant) and
     produces out[i-half].

Precision: the scores chain needs ~fp32 accuracy (softmax here is a
near-argmax; top-2 score gaps go down to ~0.06 while |scores| reaches 1.7e5),
but fp32 matmuls run at ~3.5 cyc/row on the PE and fp32r at ~2.25. fp16
streams at 1 cyc/row, so every chain tensor x is held as a hi/lo fp16 pair
(x = xh + xl, 11+11 mantissa bits) and each product uses 3 full-rate
matmuls: Ah*Bh + Ah*Bl + Al*Bh, accumulated in fp32 PSUM -- fp32-grade
products at ~3x fp16 speed. Inputs are pre-scaled by powers of two
(emb*32, W*32, AT/64) so the lo limbs stay in fp16 normal range; the exact
compensation happens in PSUM-evacuation scales and the softmax exp scale.
V and the attn@V stage are post-softmax (error passes through linearly) and
use single fp16.
"""

import math

import numpy as np

import concourse.bass as bass
import concourse.bass_utils as _bu
import concourse.mybir as mybir
import concourse.tile as tile
from concourse import bacc
from concourse.bass_utils import run_bass_kernel_spmd
from concourse.masks import make_identity

# LDWEIGHTS dedup: consecutive matmuls sharing a stationary operand skip the
# reload. Verified to produce bit-identical output on this kernel.
if not getattr(_bu, "_ldw_opt_patched", False):
    _orig_walrus_args = _bu.get_walrus_args

    def _walrus_args_ldw(arch, tmpdir, *, dve_root=None):
        args = _orig_walrus_args(arch, tmpdir, dve_root=dve_root)
        return [a.replace("--enable-ldw-opt=false", "--enable-ldw-opt=true") for a in args]

    _bu.get_walrus_args = _walrus_args_ldw
    _bu._ldw_opt_patched = True

dt = mybir.dt
P = 128
N_CORES = 8


def _split16(x):
    """x (fp32) -> (hi, lo) fp16 limbs with x ~= hi + lo (22-bit mantissa)."""
    x = np.ascontiguousarray(x, dtype=np.float32)
    hi = x.astype(np.float16)
    lo = (x - hi.astype(np.float32)).astype(np.float16)
    return hi, lo


def build_g_nc(E, H):
    """Launch 1: per-core partial G' = (32*W_k[hq])^T @ (32*W_q[hq][:, e'half]).

    Core c handles e'-half (c % 2) and h-quarter (c // 2); output is the true
    G partial (the 2^-10 prescale compensation applied at PSUM evacuation).
    """
    EH = E // 2
    HQ = H // 4
    EB = E // P
    HCB = HQ // P
    GW = min(512, EH)
    NGB = EH // GW
    f32, f16 = dt.float32, dt.float16

    nc = bacc.Bacc("TRN2", target_bir_lowering=False, debug=False)
    wkh = nc.dram_tensor("wkh", [HQ, E], f16, kind="ExternalInput").ap()
    wkl = nc.dram_tensor("wkl", [HQ, E], f16, kind="ExternalInput").ap()
    wqh = nc.dram_tensor("wqh", [HQ, EH], f16, kind="ExternalInput").ap()
    wql = nc.dram_tensor("wql", [HQ, EH], f16, kind="ExternalInput").ap()
    g_part = nc.dram_tensor("g_part", [E, EH], f32, kind="ExternalOutput").ap()

    with tile.TileContext(nc) as tc:
        with (
            tc.tile_pool(name="p_res", bufs=1) as p_res,
            tc.tile_pool(name="p_gs", bufs=3) as p_gs,
            tc.tile_pool(name="ps_g", bufs=8, space="PSUM") as ps_g,
        ):
            gp = p_res.tile([P, EB, EH], f32)
            pt_g = [
                [
                    ps_g.tile([P, GW], f32, tag="gps", name=f"gps_{eb}_{nb}")
                    for nb in range(NGB)
                ]
                for eb in range(EB)
            ]
            for hc in range(HCB):
                hs = slice(hc * P, (hc + 1) * P)
                kh = p_gs.tile([P, E], f16, tag="kh")
                nc.sync.dma_start(kh[:], wkh[hs, :])
                kl = p_gs.tile([P, E], f16, tag="kl")
                nc.sync.dma_start(kl[:], wkl[hs, :])
                qh = p_gs.tile([P, EH], f16, tag="qh")
                nc.sync.dma_start(qh[:], wqh[hs, :])
                ql = p_gs.tile([P, EH], f16, tag="ql")
                nc.sync.dma_start(ql[:], wql[hs, :])
                first, last = hc == 0, hc == HCB - 1
                for eb in range(EB):
                    ksl = slice(eb * P, (eb + 1) * P)
                    for nb in range(NGB):
                        nsl = slice(nb * GW, (nb + 1) * GW)
                        pt = pt_g[eb][nb]
                        nc.tensor.matmul(
                            pt[:], kh[:, ksl], qh[:, nsl], start=first, stop=False
                        )
                        nc.tensor.matmul(
                            pt[:], kh[:, ksl], ql[:, nsl], start=False, stop=False
                        )
                        nc.tensor.matmul(
                            pt[:], kl[:, ksl], qh[:, nsl], start=False, stop=last
                        )
            for eb in range(EB):
                for nb in range(NGB):
                    nc.vector.tensor_scalar_mul(
                        gp[:, eb, nb * GW : (nb + 1) * GW], pt_g[eb][nb][:], 2.0**-10
                    )
            nc.sync.dma_start(
                g_part.rearrange("(eo p) e2 -> p eo e2", p=P), gp[:]
            )

    nc.compile()
    return nc


def build_main_nc(S, E, H, O):
    """Launch 2: attention for one (batch, i-half); G given as fp16 limbs."""
    SI = S // 2          # i rows per core
    EB = E // P          # 128-chunks of the embedding dim
    JB = S // P
    IB = SI // P
    IW = min(512, SI)    # AT moving width along i
    NIH = SI // IW
    JW = min(512, S)     # scores moving width along j
    NJW = S // JW
    OW = min(512, O)
    NOW = O // OW
    # scores PSUM = (AT/64)*(emb*32) = raw/2 ; exp arg must be raw/sqrt(H)
    SCALE_EXP = 2.0 / math.sqrt(H)

    f32, f16 = dt.float32, dt.float16

    nc = bacc.Bacc("TRN2", target_bir_lowering=False, debug=False)
    g_h = nc.dram_tensor("g_h", [E, E], f16, kind="ExternalInput").ap()
    g_l = nc.dram_tensor("g_l", [E, E], f16, kind="ExternalInput").ap()
    et_h = nc.dram_tensor("et_h", [E, S], f16, kind="ExternalInput").ap()
    et_l = nc.dram_tensor("et_l", [E, S], f16, kind="ExternalInput").ap()
    wvt = nc.dram_tensor("wvt", [E, O], f16, kind="ExternalInput").ap()
    out = nc.dram_tensor("out", [SI, O], f32, kind="ExternalOutput").ap()

    with tile.TileContext(nc) as tc:
        with (
            tc.tile_pool(name="misc", bufs=2) as misc,
            tc.tile_pool(name="p_big", bufs=1) as p_big,
        ):
            ident = misc.tile([P, P], f16, tag="ident", name="ident")
            make_identity(nc, ident[:])

            # whole-kernel residents
            eth = p_big.tile([P, EB, S], f16)   # embT*32 hi: [e part, e chunk, tok]
            etl = p_big.tile([P, EB, S], f16)
            ath = p_big.tile([P, EB, SI], f16)  # AT/64: [e' part, e' chunk, i]
            atl = p_big.tile([P, EB, SI], f16)
            v16 = p_big.tile([P, JB, O], f16)   # V: [j part, j chunk, o]

            # i-columns first (unblock AT), then the j-extension for scores
            nc.sync.dma_start(
                eth[:, :, :SI], et_h[:, :SI].rearrange("(eo p) t -> p eo t", p=P)
            )
            nc.sync.dma_start(
                etl[:, :, :SI], et_l[:, :SI].rearrange("(eo p) t -> p eo t", p=P)
            )
            if SI < S:
                nc.sync.dma_start(
                    eth[:, :, SI:], et_h[:, SI:].rearrange("(eo p) t -> p eo t", p=P)
                )
                nc.sync.dma_start(
                    etl[:, :, SI:], et_l[:, SI:].rearrange("(eo p) t -> p eo t", p=P)
                )

            with tc.tile_pool(name="ps", bufs=8, space="PSUM") as ps:
                # ---- AT = G^T embT / 64 (hi/lo split x3) ----
                with tc.tile_pool(name="p_g", bufs=1) as p_g:
                    gh = p_g.tile([P, EB, E], f16)  # [e part, e chunk, e']
                    gl = p_g.tile([P, EB, E], f16)
                    nc.sync.dma_start(gh[:], g_h.rearrange("(eo p) e2 -> p eo e2", p=P))
                    nc.sync.dma_start(gl[:], g_l.rearrange("(eo p) e2 -> p eo e2", p=P))
                    for ih in range(NIH):
                        isl = slice(ih * IW, (ih + 1) * IW)
                        for epb in range(EB):
                            psl = slice(epb * P, (epb + 1) * P)
                            pt = ps.tile([P, IW], f32, tag="ps", name=f"aps_{ih}_{epb}")
                            for eb in range(EB):
                                first, last = eb == 0, eb == EB - 1
                                nc.tensor.matmul(
                                    pt[:], gh[:, eb, psl], eth[:, eb, isl],
                                    start=first, stop=False,
                                )
                                nc.tensor.matmul(
                                    pt[:], gh[:, eb, psl], etl[:, eb, isl],
                                    start=False, stop=False,
                                )
                                nc.tensor.matmul(
                                    pt[:], gl[:, eb, psl], eth[:, eb, isl],
                                    start=False, stop=last,
                                )
                            atmp = misc.tile([P, IW], f32, tag="atmp", name=f"atmp_{ih}_{epb}")
                            nc.vector.tensor_scalar_mul(atmp[:], pt[:], 2.0**-11)
                            nc.vector.tensor_copy(ath[:, epb, isl], atmp[:])
                            nc.vector.tensor_tensor(
                                atl[:, epb, isl], atmp[:], ath[:, epb, isl],
                                mybir.AluOpType.subtract,
                            )

                # ---- V[j, o] = (embT*32)^T WvT / 32 (single fp16) ----
                with tc.tile_pool(name="p_vw", bufs=1) as p_vw:
                    wvc = p_vw.tile([P, EB, O], f16)
                    nc.sync.dma_start(wvc[:], wvt.rearrange("(eo p) o -> p eo o", p=P))
                    for jb in range(JB):
                        jsl = slice(jb * P, (jb + 1) * P)
                        pv_tiles = [
                            ps.tile([P, OW], f32, tag="ps", name=f"vps_{jb}_{ob}")
                            for ob in range(NOW)
                        ]
                        for eb in range(EB):
                            for ob in range(NOW):
                                osl = slice(ob * OW, (ob + 1) * OW)
                                nc.tensor.matmul(
                                    pv_tiles[ob][:], eth[:, eb, jsl], wvc[:, eb, osl],
                                    start=(eb == 0), stop=(eb == EB - 1),
                                )
                        for ob in range(NOW):
                            osl = slice(ob * OW, (ob + 1) * OW)
                            nc.vector.tensor_scalar_mul(
                                v16[:, jb, osl], pv_tiles[ob][:], 2.0**-5
                            )

                # ---- scores + softmax + out, fused per 128-row i block ----
                with (
                    tc.tile_pool(name="p_sw", bufs=2) as p_sw,
                    tc.tile_pool(name="p_sw1", bufs=1) as p_sw1,
                ):
                    for ib in range(IB):
                        ibs = slice(ib * P, (ib + 1) * P)
                        pt_s = [
                            ps.tile([P, JW], f32, tag="ps", name=f"sps_{ib}_{w}")
                            for w in range(NJW)
                        ]
                        for epb in range(EB):
                            for w in range(NJW):
                                wsl = slice(w * JW, (w + 1) * JW)
                                nc.tensor.matmul(
                                    pt_s[w][:], ath[:, epb, ibs], eth[:, epb, wsl],
                                    start=(epb == 0), stop=False,
                                )
                                nc.tensor.matmul(
                                    pt_s[w][:], ath[:, epb, ibs], etl[:, epb, wsl],
                                    start=False, stop=False,
                                )
                                nc.tensor.matmul(
                                    pt_s[w][:], atl[:, epb, ibs], eth[:, epb, wsl],
                                    start=False, stop=(epb == EB - 1),
                                )
                        sc = p_sw.tile([P, S], f32, tag="sc")
                        for w in range(NJW):
                            nc.vector.tensor_copy(
                                sc[:, w * JW : (w + 1) * JW], pt_s[w][:]
                            )
                        nmx = p_sw.tile([P, 1], f32, tag="nmx")
                        nc.vector.reduce_max(
                            nmx[:], sc[:], axis=mybir.AxisListType.X, negate=True
                        )
                        nmx2 = p_sw.tile([P, 1], f32, tag="nmx2")
                        nc.vector.tensor_scalar_mul(nmx2[:], nmx[:], SCALE_EXP)
                        nc.scalar.activation(
                            sc[:], sc[:], mybir.ActivationFunctionType.Exp,
                            bias=nmx2[:], scale=SCALE_EXP,
                        )
                        sm = p_sw.tile([P, 1], f32, tag="sm")
                        nc.vector.reduce_sum(sm[:], sc[:], axis=mybir.AxisListType.X)
                        rs = p_sw.tile([P, 1], f32, tag="rs")
                        nc.vector.reciprocal(rs[:], sm[:])
                        attn16 = p_sw.tile([P, S], f16, tag="attn16")
                        nc.vector.tensor_scalar_mul(attn16[:], sc[:], rs[:])
                        attnT = p_sw1.tile([P, JB, P], f16, tag="attnT")
                        for jb in range(JB):
                            tp = ps.tile([P, P], f16, tag="ps", name=f"tps_{ib}_{jb}")
                            nc.tensor.transpose(
                                tp[:], attn16[:, jb * P : (jb + 1) * P], ident[:]
                            )
                            nc.vector.tensor_copy(attnT[:, jb, :], tp[:])
                        pt_o = [
                            ps.tile([P, OW], f32, tag="ps", name=f"ops_{ib}_{ob}")
                            for ob in range(NOW)
                        ]
                        for jb in range(JB):
                            for ob in range(NOW):
                                nc.tensor.matmul(
                                    pt_o[ob][:],
                                    attnT[:, jb, :],
                                    v16[:, jb, ob * OW : (ob + 1) * OW],
                                    start=(jb == 0), stop=(jb == JB - 1),
                                )
                        outt = p_sw1.tile([P, O], f32, tag="outt")
                        for ob in range(NOW):
                            nc.vector.tensor_copy(
                                outt[:, ob * OW : (ob + 1) * OW], pt_o[ob][:]
                            )
                        nc.sync.dma_start(out[ibs, :], outt[:])

    nc.compile()
    return nc


_NC_CACHE = {}


def _get_nc(builder, *key):
    k = (builder.__name__,) + key
    if k not in _NC_CACHE:
        _NC_CACHE[k] = builder(*key)
    return _NC_CACHE[k]


def kernel(token_emb, W_q, W_k, W_v, mask=None, _trace=False, _tmpdir=None):
    token_emb = np.asarray(token_emb, np.float32)
    W_q = np.asarray(W_q, np.float32)
    W_k = np.asarray(W_k, np.float32)
    W_v = np.asarray(W_v, np.float32)
    B, S, E = token_emb.shape
    H = W_q.shape[0]
    O = W_v.shape[0]
    SI = S // 2
    EH = E // 2
    HQ = H // 4
    assert 2 * B == N_CORES

    # ---- launch 1: G = W_k^T @ W_q, sharded (e'-half x h-quarter) ----
    nc_g = _get_nc(build_g_nc, E, H)
    wk_h, wk_l = _split16(W_k * 32.0)
    wq_h, wq_l = _split16(W_q * 32.0)
    g_maps = []
    for c in range(N_CORES):
        half, hq = c % 2, c // 2
        hsl = slice(hq * HQ, (hq + 1) * HQ)
        esl = slice(half * EH, (half + 1) * EH)
        g_maps.append(
            {
                "wkh": np.ascontiguousarray(wk_h[hsl]),
                "wkl": np.ascontiguousarray(wk_l[hsl]),
                "wqh": np.ascontiguousarray(wq_h[hsl, esl]),
                "wql": np.ascontiguousarray(wq_l[hsl, esl]),
            }
        )
    res_g = run_bass_kernel_spmd(
        nc_g, g_maps, core_ids=list(range(N_CORES)), trace=_trace,
        tmpdir=(_tmpdir + "/g" if _tmpdir else None),
    )
    G = np.empty((E, E), np.float32)
    for half in range(2):
        esl = slice(half * EH, (half + 1) * EH)
        G[:, esl] = sum(
            res_g.results[2 * q + half]["g_part"].astype(np.float64)
            for q in range(4)
        ).astype(np.float32)
    g_h, g_l = _split16(G)

    # ---- launch 2: attention ----
    nc_main = _get_nc(build_main_nc, S, E, H, O)
    wvt = np.ascontiguousarray(W_v.T).astype(np.float16)
    in_maps = []
    for c in range(N_CORES):
        b, half = divmod(c, 2)
        e = token_emb[b]
        perm = np.concatenate(
            [e[half * SI : (half + 1) * SI], e[(1 - half) * SI : (2 - half) * SI]],
            axis=0,
        )
        et_h, et_l = _split16(perm.T * 32.0)
        in_maps.append(
            {
                "g_h": g_h, "g_l": g_l, "et_h": et_h, "et_l": et_l, "wvt": wvt,
            }
        )
    res = run_bass_kernel_spmd(
        nc_main, in_maps, core_ids=list(range(N_CORES)), trace=_trace,
        tmpdir=(_tmpdir + "/main" if _tmpdir else None),
    )

    out = np.empty((B, S, O), np.float32)
    for c in range(N_CORES):
        b, half = divmod(c, 2)
        out[b, half * SI : (half + 1) * SI] = res.results[c]["out"]
    if _trace:
        kernel._last_results = (res_g, res)
    return out


# revision 10
# speedup vs baseline: 2.0016x; 1.1969x over previous
"""CavemanGPT single-head attention on 8 Trainium2 NeuronCores.

Math (reference; its mask input is unused there):
    Q = emb @ W_q^T ; K = emb @ W_k^T ; V = emb @ W_v^T        (per batch b)
    out = softmax(K @ Q^T / sqrt(H), axis=-1) @ V

Key algebraic restructure: K @ Q^T = emb @ (W_k^T W_q) @ emb^T, so with
G := W_k^T @ W_q  ([E, E], batch independent) the per-core work drops from
~52 GFLOP to ~16 GFLOP and the giant [S, H] Q/K intermediates vanish:
    AT := (G^T @ emb_i^T) / 64     ([E, SI])
    scores = AT^T @ emb^T          ([SI, S], = true scores / 2)
    out = softmax(...) @ V

Two launches:
  1. G-launch: G = W_k^T @ W_q sharded over 8 cores (2 e'-halves x 4
     h-quarters); host sums the h-partials (in fp64).
  2. Main launch: 8 cores = 4 batches x 2 halves of the i (output-row)
     dimension. Each core receives its batch's emb with its own i-half
     permuted to the front (softmax over j is permutation invariant) and
     produces out[i-half].

Precision: the scores chain needs ~fp32 accuracy (softmax here is a
near-argmax; top-2 score gaps go down to ~0.06 while |scores| reaches 1.7e5),
but fp32 matmuls run at ~3.5 cyc/row on the PE and fp32r at ~2.25. fp16
streams at 1 cyc/row, so every chain tensor x is held as a hi/lo fp16 pair
(x = xh + xl, 11+11 mantissa bits) and each product uses 3 full-rate
matmuls: Ah*Bh + Ah*Bl + Al*Bh, accumulated in fp32 PSUM -- fp32-grade
products at ~3x fp16 speed. Inputs are pre-scaled by powers of two
(emb*32, W*32, AT/64) so the lo limbs stay in fp16 normal range; the exact
compensation happens in PSUM-evacuation scales and the softmax exp scale.
V and the attn@V stage are post-softmax (error passes through linearly) and
use single fp16.
"""

import math

import numpy as np

import concourse.bass as bass
import concourse.bass_utils as _bu
import concourse.mybir as mybir
import concourse.tile as tile
from concourse import bacc
from concourse.bass_utils import run_bass_kernel_spmd
from concourse.masks import make_identity

# LDWEIGHTS dedup: consecutive matmuls sharing a stationary operand skip the
# reload. Verified to produce bit-identical output on this kernel.
if not getattr(_bu, "_ldw_opt_patched", False):
    _orig_walrus_args = _bu.get_walrus_args

    def _walrus_args_ldw(arch, tmpdir, *, dve_root=None):
        args = _orig_walrus_args(arch, tmpdir, dve_root=dve_root)
        return [a.replace("--enable-ldw-opt=false", "--enable-ldw-opt=true") for a in args]

    _bu.get_walrus_args = _walrus_args_ldw
    _bu._ldw_opt_patched = True

dt = mybir.dt
P = 128
N_CORES = 8


def _split16(x):
    """x (fp32) -> (hi, lo) fp16 limbs with x ~= hi + lo (22-bit mantissa)."""
    x = np.ascontiguousarray(x, dtype=np.float32)
    hi = x.astype(np.float16)
    lo = (x - hi.astype(np.float32)).astype(np.float16)
    return hi, lo


def build_g_nc(E, H):
    """Launch 1: per-core partial G' = (32*W_k[hq])^T @ (32*W_q[hq][:, e'half]).

    Core c handles e'-half (c % 2) and h-quarter (c // 2); output is the true
    G partial (the 2^-10 prescale compensation applied at PSUM evacuation).
    """
    EH = E // 2
    HQ = H // 4
    EB = E // P
    HCB = HQ // P
    GW = min(512, EH)
    NGB = EH // GW
    f32, f16 = dt.float32, dt.float16

    nc = bacc.Bacc("TRN2", target_bir_lowering=False, debug=False)
    wkh = nc.dram_tensor("wkh", [HQ, E], f16, kind="ExternalInput").ap()
    wkl = nc.dram_tensor("wkl", [HQ, E], f16, kind="ExternalInput").ap()
    wqh = nc.dram_tensor("wqh", [HQ, EH], f16, kind="ExternalInput").ap()
    wql = nc.dram_tensor("wql", [HQ, EH], f16, kind="ExternalInput").ap()
    g_part = nc.dram_tensor("g_part", [E, EH], f32, kind="ExternalOutput").ap()

    with tile.TileContext(nc) as tc:
        with (
            tc.tile_pool(name="p_res", bufs=1) as p_res,
            tc.tile_pool(name="p_gs", bufs=3) as p_gs,
            tc.tile_pool(name="ps_g", bufs=8, space="PSUM") as ps_g,
        ):
            gp = p_res.tile([P, EB, EH], f32)
            pt_g = [
                [
                    ps_g.tile([P, GW], f32, tag="gps", name=f"gps_{eb}_{nb}")
                    for nb in range(NGB)
                ]
                for eb in range(EB)
            ]
            for hc in range(HCB):
                hs = slice(hc * P, (hc + 1) * P)
                kh = p_gs.tile([P, E], f16, tag="kh")
                nc.sync.dma_start(kh[:], wkh[hs, :])
                kl = p_gs.tile([P, E], f16, tag="kl")
                nc.sync.dma_start(kl[:], wkl[hs, :])
                qh = p_gs.tile([P, EH], f16, tag="qh")
                nc.sync.dma_start(qh[:], wqh[hs, :])
                ql = p_gs.tile([P, EH], f16, tag="ql")
                nc.sync.dma_start(ql[:], wql[hs, :])
                first, last = hc == 0, hc == HCB - 1
                for eb in range(EB):
                    ksl = slice(eb * P, (eb + 1) * P)
                    for nb in range(NGB):
                        nsl = slice(nb * GW, (nb + 1) * GW)
                        pt = pt_g[eb][nb]
                        nc.tensor.matmul(
                            pt[:], kh[:, ksl], qh[:, nsl], start=first, stop=False
                        )
                        nc.tensor.matmul(
                            pt[:], kh[:, ksl], ql[:, nsl], start=False, stop=False
                        )
                        nc.tensor.matmul(
                            pt[:], kl[:, ksl], qh[:, nsl], start=False, stop=last
                        )
            for eb in range(EB):
                for nb in range(NGB):
                    nc.vector.tensor_scalar_mul(
                        gp[:, eb, nb * GW : (nb + 1) * GW], pt_g[eb][nb][:], 2.0**-10
                    )
            nc.sync.dma_start(
                g_part.rearrange("(eo p) e2 -> p eo e2", p=P), gp[:]
            )

    nc.compile()
    return nc


def build_main_nc(S, E, H, O):
    """Launch 2: attention for one (batch, i-half); G given as fp16 limbs."""
    SI = S // 2          # i rows per core
    EB = E // P          # 128-chunks of the embedding dim
    JB = S // P
    IB = SI // P
    IW = min(512, SI)    # AT moving width along i
    NIH = SI // IW
    JW = min(512, S)     # scores moving width along j
    NJW = S // JW
    OW = min(512, O)
    NOW = O // OW
    # scores PSUM = (AT/64)*(emb*32) = raw/2 ; exp arg must be raw/sqrt(H)
    SCALE_EXP = 2.0 / math.sqrt(H)

    f32, f16 = dt.float32, dt.float16

    nc = bacc.Bacc("TRN2", target_bir_lowering=False, debug=False)
    g_h = nc.dram_tensor("g_h", [E, E], f16, kind="ExternalInput").ap()
    g_l = nc.dram_tensor("g_l", [E, E], f16, kind="ExternalInput").ap()
    et_h = nc.dram_tensor("et_h", [E, S], f16, kind="ExternalInput").ap()
    et_l = nc.dram_tensor("et_l", [E, S], f16, kind="ExternalInput").ap()
    wvt = nc.dram_tensor("wvt", [E, O], f16, kind="ExternalInput").ap()
    out = nc.dram_tensor("out", [SI, O], f32, kind="ExternalOutput").ap()

    with tile.TileContext(nc) as tc:
        with (
            tc.tile_pool(name="misc", bufs=2) as misc,
            tc.tile_pool(name="p_big", bufs=1) as p_big,
        ):
            ident = misc.tile([P, P], f16, tag="ident", name="ident")
            make_identity(nc, ident[:])

            # whole-kernel residents
            eth = p_big.tile([P, EB, S], f16)   # embT*32 hi: [e part, e chunk, tok]
            etl = p_big.tile([P, EB, S], f16)
            ath = p_big.tile([P, EB, SI], f16)  # AT/64: [e' part, e' chunk, i]
            atl = p_big.tile([P, EB, SI], f16)
            v16 = p_big.tile([P, JB, O], f16)   # V: [j part, j chunk, o]
            wvc = p_big.tile([P, EB, O], f16)   # WvT: [e part, e chunk, o]

            nc.sync.dma_start(wvc[:], wvt.rearrange("(eo p) o -> p eo o", p=P))
            # i-columns first (unblock AT), then the j-extension for scores
            nc.sync.dma_start(
                eth[:, :, :SI], et_h[:, :SI].rearrange("(eo p) t -> p eo t", p=P)
            )
            nc.sync.dma_start(
                etl[:, :, :SI], et_l[:, :SI].rearrange("(eo p) t -> p eo t", p=P)
            )
            if SI < S:
                nc.sync.dma_start(
                    eth[:, :, SI:], et_h[:, SI:].rearrange("(eo p) t -> p eo t", p=P)
                )
                nc.sync.dma_start(
                    etl[:, :, SI:], et_l[:, SI:].rearrange("(eo p) t -> p eo t", p=P)
                )

            with tc.tile_pool(name="ps", bufs=8, space="PSUM") as ps:
                # ---- AT = G^T embT / 64 (hi/lo split x3) ----
                with tc.tile_pool(name="p_g", bufs=1) as p_g:
                    gh = p_g.tile([P, EB, E], f16)  # [e part, e chunk, e']
                    gl = p_g.tile([P, EB, E], f16)
                    nc.sync.dma_start(gh[:], g_h.rearrange("(eo p) e2 -> p eo e2", p=P))
                    nc.sync.dma_start(gl[:], g_l.rearrange("(eo p) e2 -> p eo e2", p=P))
                    for ih in range(NIH):
                        isl = slice(ih * IW, (ih + 1) * IW)
                        for epb in range(EB):
                            psl = slice(epb * P, (epb + 1) * P)
                            pt = ps.tile([P, IW], f32, tag="ps", name=f"aps_{ih}_{epb}")
                            for eb in range(EB):
                                first, last = eb == 0, eb == EB - 1
                                nc.tensor.matmul(
                                    pt[:], gh[:, eb, psl], eth[:, eb, isl],
                                    start=first, stop=False,
                                )
                                nc.tensor.matmul(
                                    pt[:], gh[:, eb, psl], etl[:, eb, isl],
                                    start=False, stop=False,
                                )
                                nc.tensor.matmul(
                                    pt[:], gl[:, eb, psl], eth[:, eb, isl],
                                    start=False, stop=last,
                                )
                            atmp = misc.tile([P, IW], f32, tag="atmp", name=f"atmp_{ih}_{epb}")
                            nc.vector.tensor_scalar_mul(atmp[:], pt[:], 2.0**-11)
                            nc.vector.tensor_copy(ath[:, epb, isl], atmp[:])
                            nc.vector.tensor_tensor(
                                atl[:, epb, isl], atmp[:], ath[:, epb, isl],
                                mybir.AluOpType.subtract,
                            )

                # ---- V[j, o] = (embT*32)^T WvT / 32 (single fp16) ----
                if True:
                    for jb in range(JB):
                        jsl = slice(jb * P, (jb + 1) * P)
                        pv_tiles = [
                            ps.tile([P, OW], f32, tag="ps", name=f"vps_{jb}_{ob}")
                            for ob in range(NOW)
                        ]
                        for eb in range(EB):
                            for ob in range(NOW):
                                osl = slice(ob * OW, (ob + 1) * OW)
                                nc.tensor.matmul(
                                    pv_tiles[ob][:], eth[:, eb, jsl], wvc[:, eb, osl],
                                    start=(eb == 0), stop=(eb == EB - 1),
                                )
                        for ob in range(NOW):
                            osl = slice(ob * OW, (ob + 1) * OW)
                            nc.vector.tensor_scalar_mul(
                                v16[:, jb, osl], pv_tiles[ob][:], 2.0**-5
                            )

                # ---- scores + softmax + out, fused per 128-row i block ----
                with (
                    tc.tile_pool(name="p_sw", bufs=2) as p_sw,
                    tc.tile_pool(name="p_sw1", bufs=2) as p_sw1,
                ):
                    def emit_scores(ib):
                        ibs = slice(ib * P, (ib + 1) * P)
                        pt_s = [
                            ps.tile([P, JW], f32, tag="ps", name=f"sps_{ib}_{w}")
                            for w in range(NJW)
                        ]
                        for epb in range(EB):
                            for w in range(NJW):
                                wsl = slice(w * JW, (w + 1) * JW)
                                nc.tensor.matmul(
                                    pt_s[w][:], ath[:, epb, ibs], eth[:, epb, wsl],
                                    start=(epb == 0), stop=False,
                                )
                                nc.tensor.matmul(
                                    pt_s[w][:], ath[:, epb, ibs], etl[:, epb, wsl],
                                    start=False, stop=False,
                                )
                                nc.tensor.matmul(
                                    pt_s[w][:], atl[:, epb, ibs], eth[:, epb, wsl],
                                    start=False, stop=(epb == EB - 1),
                                )
                        return pt_s

                    pt_s = emit_scores(0)
                    for ib in range(IB):
                        ibs = slice(ib * P, (ib + 1) * P)
                        sc = p_sw.tile([P, S], f32, tag="sc")
                        for w in range(NJW):
                            nc.vector.tensor_copy(
                                sc[:, w * JW : (w + 1) * JW], pt_s[w][:]
                            )
                        nmx = p_sw.tile([P, 1], f32, tag="nmx")
                        nc.vector.reduce_max(
                            nmx[:], sc[:], axis=mybir.AxisListType.X, negate=True
                        )
                        nmx2 = p_sw.tile([P, 1], f32, tag="nmx2")
                        nc.vector.tensor_scalar_mul(nmx2[:], nmx[:], SCALE_EXP)
                        nc.scalar.activation(
                            sc[:], sc[:], mybir.ActivationFunctionType.Exp,
                            bias=nmx2[:], scale=SCALE_EXP,
                        )
                        sm = p_sw.tile([P, 1], f32, tag="sm")
                        nc.vector.reduce_sum(sm[:], sc[:], axis=mybir.AxisListType.X)
                        rs = p_sw.tile([P, 1], f32, tag="rs")
                        nc.vector.reciprocal(rs[:], sm[:])
                        attn16 = p_sw.tile([P, S], f16, tag="attn16")
                        nc.vector.tensor_scalar_mul(attn16[:], sc[:], rs[:])
                        if ib + 1 < IB:
                            pt_s = emit_scores(ib + 1)
                        attnT = p_sw1.tile([P, JB, P], f16, tag="attnT")
                        for jb in range(JB):
                            tp = ps.tile([P, P], f16, tag="ps", name=f"tps_{ib}_{jb}")
                            nc.tensor.transpose(
                                tp[:], attn16[:, jb * P : (jb + 1) * P], ident[:]
                            )
                            nc.vector.tensor_copy(attnT[:, jb, :], tp[:])
                        pt_o = [
                            ps.tile([P, OW], f32, tag="ps", name=f"ops_{ib}_{ob}")
                            for ob in range(NOW)
                        ]
                        for jb in range(JB):
                            for ob in range(NOW):
                                nc.tensor.matmul(
                                    pt_o[ob][:],
                                    attnT[:, jb, :],
                                    v16[:, jb, ob * OW : (ob + 1) * OW],
                                    start=(jb == 0), stop=(jb == JB - 1),
                                )
                        outt = p_sw1.tile([P, O], f32, tag="outt")
                        for ob in range(NOW):
                            nc.vector.tensor_copy(
                                outt[:, ob * OW : (ob + 1) * OW], pt_o[ob][:]
                            )
                        nc.sync.dma_start(out[ibs, :], outt[:])

    nc.compile()
    return nc


_NC_CACHE = {}


def _get_nc(builder, *key):
    k = (builder.__name__,) + key
    if k not in _NC_CACHE:
        _NC_CACHE[k] = builder(*key)
    return _NC_CACHE[k]


def kernel(token_emb, W_q, W_k, W_v, mask=None, _trace=False, _tmpdir=None):
    token_emb = np.asarray(token_emb, np.float32)
    W_q = np.asarray(W_q, np.float32)
    W_k = np.asarray(W_k, np.float32)
    W_v = np.asarray(W_v, np.float32)
    B, S, E = token_emb.shape
    H = W_q.shape[0]
    O = W_v.shape[0]
    SI = S // 2
    EH = E // 2
    HQ = H // 4
    assert 2 * B == N_CORES

    # ---- launch 1: G = W_k^T @ W_q, sharded (e'-half x h-quarter) ----
    nc_g = _get_nc(build_g_nc, E, H)
    wk_h, wk_l = _split16(W_k * 32.0)
    wq_h, wq_l = _split16(W_q * 32.0)
    g_maps = []
    for c in range(N_CORES):
        half, hq = c % 2, c // 2
        hsl = slice(hq * HQ, (hq + 1) * HQ)
        esl = slice(half * EH, (half + 1) * EH)
        g_maps.append(
            {
                "wkh": np.ascontiguousarray(wk_h[hsl]),
                "wkl": np.ascontiguousarray(wk_l[hsl]),
                "wqh": np.ascontiguousarray(wq_h[hsl, esl]),
                "wql": np.ascontiguousarray(wq_l[hsl, esl]),
            }
        )
    res_g = run_bass_kernel_spmd(
        nc_g, g_maps, core_ids=list(range(N_CORES)), trace=_trace,
        tmpdir=(_tmpdir + "/g" if _tmpdir else None),
    )
    G = np.empty((E, E), np.float32)
    for half in range(2):
        esl = slice(half * EH, (half + 1) * EH)
        G[:, esl] = sum(
            res_g.results[2 * q + half]["g_part"].astype(np.float64)
            for q in range(4)
        ).astype(np.float32)
    g_h, g_l = _split16(G)

    # ---- launch 2: attention ----
    nc_main = _get_nc(build_main_nc, S, E, H, O)
    wvt = np.ascontiguousarray(W_v.T).astype(np.float16)
    in_maps = []
    for c in range(N_CORES):
        b, half = divmod(c, 2)
        e = token_emb[b]
        perm = np.concatenate(
            [e[half * SI : (half + 1) * SI], e[(1 - half) * SI : (2 - half) * SI]],
            axis=0,
        )
        et_h, et_l = _split16(perm.T * 32.0)
        in_maps.append(
            {
                "g_h": g_h, "g_l": g_l, "et_h": et_h, "et_l": et_l, "wvt": wvt,
            }
        )
    res = run_bass_kernel_spmd(
        nc_main, in_maps, core_ids=list(range(N_CORES)), trace=_trace,
        tmpdir=(_tmpdir + "/main" if _tmpdir else None),
    )

    out = np.empty((B, S, O), np.float32)
    for c in range(N_CORES):
        b, half = divmod(c, 2)
        out[b, half * SI : (half + 1) * SI] = res.results[c]["out"]
    if _trace:
        kernel._last_results = (res_g, res)
    return out


# revision 11
# speedup vs baseline: 2.1318x; 1.0650x over previous
"""CavemanGPT single-head attention on 8 Trainium2 NeuronCores.

Math (reference; its mask input is unused there):
    Q = emb @ W_q^T ; K = emb @ W_k^T ; V = emb @ W_v^T        (per batch b)
    out = softmax(K @ Q^T / sqrt(H), axis=-1) @ V

Key algebraic restructure: K @ Q^T = emb @ (W_k^T W_q) @ emb^T, so with
G := W_k^T @ W_q  ([E, E], batch independent) the per-core work drops from
~52 GFLOP to ~16 GFLOP and the giant [S, H] Q/K intermediates vanish:
    AT := (G^T @ emb_i^T) / 64     ([E, SI])
    scores = AT^T @ emb^T          ([SI, S], = true scores / 2)
    out = softmax(...) @ V

Two launches:
  1. G-launch: G = W_k^T @ W_q sharded over 8 cores (2 e'-halves x 4
     h-quarters); host sums the h-partials (in fp64).
  2. Main launch: 8 cores = 4 batches x 2 halves of the i (output-row)
     dimension. Each core receives its batch's emb with its own i-half
     permuted to the front (softmax over j is permutation invariant) and
     produces out[i-half].

Precision: the scores chain needs ~fp32 accuracy (softmax here is a
near-argmax; top-2 score gaps go down to ~0.06 while |scores| reaches 1.7e5),
but fp32 matmuls run at ~3.5 cyc/row on the PE and fp32r at ~2.25. fp16
streams at 1 cyc/row, so every chain tensor x is held as a hi/lo fp16 pair
(x = xh + xl, 11+11 mantissa bits) and each product uses 3 full-rate
matmuls: Ah*Bh + Ah*Bl + Al*Bh, accumulated in fp32 PSUM -- fp32-grade
products at ~3x fp16 speed. Inputs are pre-scaled by powers of two
(emb*32, W*32, AT/64) so the lo limbs stay in fp16 normal range; the exact
compensation happens in PSUM-evacuation scales and the softmax exp scale.
V and the attn@V stage are post-softmax (error passes through linearly) and
use single fp16.
"""

import math

import numpy as np

import concourse.bass as bass
import concourse.bass_utils as _bu
import concourse.mybir as mybir
import concourse.tile as tile
from concourse import bacc
from concourse.bass_utils import run_bass_kernel_spmd
from concourse.masks import make_identity

# LDWEIGHTS dedup: consecutive matmuls sharing a stationary operand skip the
# reload. Verified to produce bit-identical output on this kernel.
if not getattr(_bu, "_ldw_opt_patched", False):
    _orig_walrus_args = _bu.get_walrus_args

    def _walrus_args_ldw(arch, tmpdir, *, dve_root=None):
        args = _orig_walrus_args(arch, tmpdir, dve_root=dve_root)
        return [a.replace("--enable-ldw-opt=false", "--enable-ldw-opt=true") for a in args]

    _bu.get_walrus_args = _walrus_args_ldw
    _bu._ldw_opt_patched = True

dt = mybir.dt
P = 128
N_CORES = 8


def _split16(x):
    """x (fp32) -> (hi, lo) fp16 limbs with x ~= hi + lo (22-bit mantissa)."""
    x = np.ascontiguousarray(x, dtype=np.float32)
    hi = x.astype(np.float16)
    lo = (x - hi.astype(np.float32)).astype(np.float16)
    return hi, lo


def build_g_nc(E, H):
    """Launch 1: per-core partial G' = (32*W_k[hq])^T @ (32*W_q[hq][:, e'half]).

    Core c handles e'-half (c % 2) and h-quarter (c // 2); output is the true
    G partial (the 2^-10 prescale compensation applied at PSUM evacuation).
    """
    EH = E // 2
    HQ = H // 4
    EB = E // P
    HCB = HQ // P
    GW = min(512, EH)
    NGB = EH // GW
    f32, f16 = dt.float32, dt.float16

    nc = bacc.Bacc("TRN2", target_bir_lowering=False, debug=False)
    wkh = nc.dram_tensor("wkh", [HQ, E], f16, kind="ExternalInput").ap()
    wkl = nc.dram_tensor("wkl", [HQ, E], f16, kind="ExternalInput").ap()
    wqh = nc.dram_tensor("wqh", [HQ, EH], f16, kind="ExternalInput").ap()
    wql = nc.dram_tensor("wql", [HQ, EH], f16, kind="ExternalInput").ap()
    g_part = nc.dram_tensor("g_part", [E, EH], f32, kind="ExternalOutput").ap()

    with tile.TileContext(nc) as tc:
        with (
            tc.tile_pool(name="p_res", bufs=1) as p_res,
            tc.tile_pool(name="p_gs", bufs=3) as p_gs,
            tc.tile_pool(name="ps_g", bufs=8, space="PSUM") as ps_g,
        ):
            gp = p_res.tile([P, EB, EH], f32)
            pt_g = [
                [
                    ps_g.tile([P, GW], f32, tag="gps", name=f"gps_{eb}_{nb}")
                    for nb in range(NGB)
                ]
                for eb in range(EB)
            ]
            for hc in range(HCB):
                hs = slice(hc * P, (hc + 1) * P)
                kh = p_gs.tile([P, E], f16, tag="kh")
                nc.sync.dma_start(kh[:], wkh[hs, :])
                kl = p_gs.tile([P, E], f16, tag="kl")
                nc.sync.dma_start(kl[:], wkl[hs, :])
                qh = p_gs.tile([P, EH], f16, tag="qh")
                nc.sync.dma_start(qh[:], wqh[hs, :])
                ql = p_gs.tile([P, EH], f16, tag="ql")
                nc.sync.dma_start(ql[:], wql[hs, :])
                first, last = hc == 0, hc == HCB - 1
                for eb in range(EB):
                    ksl = slice(eb * P, (eb + 1) * P)
                    for nb in range(NGB):
                        nsl = slice(nb * GW, (nb + 1) * GW)
                        pt = pt_g[eb][nb]
                        nc.tensor.matmul(
                            pt[:], kh[:, ksl], qh[:, nsl], start=first, stop=False
                        )
                        nc.tensor.matmul(
                            pt[:], kh[:, ksl], ql[:, nsl], start=False, stop=False
                        )
                        nc.tensor.matmul(
                            pt[:], kl[:, ksl], qh[:, nsl], start=False, stop=last
                        )
            for eb in range(EB):
                for nb in range(NGB):
                    nc.vector.tensor_scalar_mul(
                        gp[:, eb, nb * GW : (nb + 1) * GW], pt_g[eb][nb][:], 2.0**-10
                    )
            nc.sync.dma_start(
                g_part.rearrange("(eo p) e2 -> p eo e2", p=P), gp[:]
            )

    nc.compile()
    return nc


def build_main_nc(S, E, H, O):
    """Launch 2: attention for one (batch, i-half); G given as fp16 limbs."""
    SI = S // 2          # i rows per core
    EB = E // P          # 128-chunks of the embedding dim
    JB = S // P
    IB = SI // P
    IW = min(512, SI)    # AT moving width along i
    NIH = SI // IW
    JW = min(512, S)     # scores moving width along j
    NJW = S // JW
    OW = min(512, O)
    NOW = O // OW
    # scores PSUM = (AT/64)*(emb*32) = raw/2 ; exp arg must be raw/sqrt(H)
    SCALE_EXP = 2.0 / math.sqrt(H)

    f32, f16 = dt.float32, dt.float16

    nc = bacc.Bacc("TRN2", target_bir_lowering=False, debug=False)
    g_h = nc.dram_tensor("g_h", [E, E], f16, kind="ExternalInput").ap()
    g_l = nc.dram_tensor("g_l", [E, E], f16, kind="ExternalInput").ap()
    et_h = nc.dram_tensor("et_h", [E, S], f16, kind="ExternalInput").ap()
    et_l = nc.dram_tensor("et_l", [E, S], f16, kind="ExternalInput").ap()
    wvt = nc.dram_tensor("wvt", [E, O], f16, kind="ExternalInput").ap()
    out = nc.dram_tensor("out", [SI, O], f32, kind="ExternalOutput").ap()

    with tile.TileContext(nc) as tc:
        with (
            tc.tile_pool(name="misc", bufs=2) as misc,
            tc.tile_pool(name="p_big", bufs=1) as p_big,
        ):
            ident = misc.tile([P, P], f16, tag="ident", name="ident")
            make_identity(nc, ident[:])

            # whole-kernel residents
            eth = p_big.tile([P, EB, S], f16)   # embT*32 hi: [e part, e chunk, tok]
            etl = p_big.tile([P, EB, S], f16)
            ath = p_big.tile([P, EB, SI], f16)  # AT/64: [e' part, e' chunk, i]
            atl = p_big.tile([P, EB, SI], f16)
            v16 = p_big.tile([P, JB, O], f16)   # V: [j part, j chunk, o]
            wvc = p_big.tile([P, EB, O], f16)   # WvT: [e part, e chunk, o]


            with tc.tile_pool(name="ps", bufs=8, space="PSUM") as ps:
                # ---- AT = G^T embT / 64 (hi/lo split x3) ----
                with tc.tile_pool(name="p_g", bufs=1) as p_g:
                    gh = p_g.tile([P, EB, E], f16)  # [e part, e chunk, e']
                    gl = p_g.tile([P, EB, E], f16)
                    # DMAs emitted in first-use order, chunked per e-block so
                    # the first AT matmuls start after ~384KB instead of 14MB.
                    ghr = g_h.rearrange("(eo p) e2 -> p eo e2", p=P)
                    glr = g_l.rearrange("(eo p) e2 -> p eo e2", p=P)
                    ethr = et_h.rearrange("(eo p) t -> p eo t", p=P)
                    etlr = et_l.rearrange("(eo p) t -> p eo t", p=P)
                    for eb in range(EB):
                        nc.sync.dma_start(gh[:, eb], ghr[:, eb])
                        nc.sync.dma_start(eth[:, eb, :SI], ethr[:, eb, :SI])
                        nc.sync.dma_start(gl[:, eb], glr[:, eb])
                        nc.sync.dma_start(etl[:, eb, :SI], etlr[:, eb, :SI])
                    nc.sync.dma_start(wvc[:], wvt.rearrange("(eo p) o -> p eo o", p=P))
                    if SI < S:
                        nc.sync.dma_start(eth[:, :, SI:], ethr[:, :, SI:])
                        nc.sync.dma_start(etl[:, :, SI:], etlr[:, :, SI:])
                    for ih in range(NIH):
                        isl = slice(ih * IW, (ih + 1) * IW)
                        pts = [
                            ps.tile([P, IW], f32, tag="ps", name=f"aps_{ih}_{epb}")
                            for epb in range(EB)
                        ]
                        for eb in range(EB):
                            first, last = eb == 0, eb == EB - 1
                            for epb in range(EB):
                                psl = slice(epb * P, (epb + 1) * P)
                                pt = pts[epb]
                                nc.tensor.matmul(
                                    pt[:], gh[:, eb, psl], eth[:, eb, isl],
                                    start=first, stop=False,
                                )
                                nc.tensor.matmul(
                                    pt[:], gh[:, eb, psl], etl[:, eb, isl],
                                    start=False, stop=False,
                                )
                                nc.tensor.matmul(
                                    pt[:], gl[:, eb, psl], eth[:, eb, isl],
                                    start=False, stop=last,
                                )
                        for epb in range(EB):
                            psl = slice(epb * P, (epb + 1) * P)
                            pt = pts[epb]
                            atmp = misc.tile([P, IW], f32, tag="atmp", name=f"atmp_{ih}_{epb}")
                            nc.vector.tensor_scalar_mul(atmp[:], pt[:], 2.0**-11)
                            nc.vector.tensor_copy(ath[:, epb, isl], atmp[:])
                            nc.vector.tensor_tensor(
                                atl[:, epb, isl], atmp[:], ath[:, epb, isl],
                                mybir.AluOpType.subtract,
                            )

                # ---- V[j, o] = (embT*32)^T WvT / 32 (single fp16) ----
                if True:
                    for jb in range(JB):
                        jsl = slice(jb * P, (jb + 1) * P)
                        pv_tiles = [
                            ps.tile([P, OW], f32, tag="ps", name=f"vps_{jb}_{ob}")
                            for ob in range(NOW)
                        ]
                        for eb in range(EB):
                            for ob in range(NOW):
                                osl = slice(ob * OW, (ob + 1) * OW)
                                nc.tensor.matmul(
                                    pv_tiles[ob][:], eth[:, eb, jsl], wvc[:, eb, osl],
                                    start=(eb == 0), stop=(eb == EB - 1),
                                )
                        for ob in range(NOW):
                            osl = slice(ob * OW, (ob + 1) * OW)
                            nc.vector.tensor_scalar_mul(
                                v16[:, jb, osl], pv_tiles[ob][:], 2.0**-5
                            )

                # ---- scores + softmax + out, fused per 128-row i block ----
                with (
                    tc.tile_pool(name="p_sw", bufs=2) as p_sw,
                    tc.tile_pool(name="p_sw1", bufs=2) as p_sw1,
                ):
                    def emit_scores(ib):
                        ibs = slice(ib * P, (ib + 1) * P)
                        pt_s = [
                            ps.tile([P, JW], f32, tag="ps", name=f"sps_{ib}_{w}")
                            for w in range(NJW)
                        ]
                        for epb in range(EB):
                            for w in range(NJW):
                                wsl = slice(w * JW, (w + 1) * JW)
                                nc.tensor.matmul(
                                    pt_s[w][:], ath[:, epb, ibs], eth[:, epb, wsl],
                                    start=(epb == 0), stop=False,
                                )
                                nc.tensor.matmul(
                                    pt_s[w][:], ath[:, epb, ibs], etl[:, epb, wsl],
                                    start=False, stop=False,
                                )
                                nc.tensor.matmul(
                                    pt_s[w][:], atl[:, epb, ibs], eth[:, epb, wsl],
                                    start=False, stop=(epb == EB - 1),
                                )
                        return pt_s

                    pt_s = emit_scores(0)
                    for ib in range(IB):
                        ibs = slice(ib * P, (ib + 1) * P)
                        sc = p_sw.tile([P, S], f32, tag="sc")
                        for w in range(NJW):
                            nc.vector.tensor_copy(
                                sc[:, w * JW : (w + 1) * JW], pt_s[w][:]
                            )
                        nmx = p_sw.tile([P, 1], f32, tag="nmx")
                        nc.vector.reduce_max(
                            nmx[:], sc[:], axis=mybir.AxisListType.X, negate=True
                        )
                        nmx2 = p_sw.tile([P, 1], f32, tag="nmx2")
                        nc.vector.tensor_scalar_mul(nmx2[:], nmx[:], SCALE_EXP)
                        nc.scalar.activation(
                            sc[:], sc[:], mybir.ActivationFunctionType.Exp,
                            bias=nmx2[:], scale=SCALE_EXP,
                        )
                        sm = p_sw.tile([P, 1], f32, tag="sm")
                        nc.vector.reduce_sum(sm[:], sc[:], axis=mybir.AxisListType.X)
                        rs = p_sw.tile([P, 1], f32, tag="rs")
                        nc.vector.reciprocal(rs[:], sm[:])
                        attn16 = p_sw.tile([P, S], f16, tag="attn16")
                        nc.vector.tensor_scalar_mul(attn16[:], sc[:], rs[:])
                        if ib + 1 < IB:
                            pt_s = emit_scores(ib + 1)
                        attnT = p_sw1.tile([P, JB, P], f16, tag="attnT")
                        for jb in range(JB):
                            tp = ps.tile([P, P], f16, tag="ps", name=f"tps_{ib}_{jb}")
                            nc.tensor.transpose(
                                tp[:], attn16[:, jb * P : (jb + 1) * P], ident[:]
                            )
                            nc.vector.tensor_copy(attnT[:, jb, :], tp[:])
                        pt_o = [
                            ps.tile([P, OW], f32, tag="ps", name=f"ops_{ib}_{ob}")
                            for ob in range(NOW)
                        ]
                        for jb in range(JB):
                            for ob in range(NOW):
                                nc.tensor.matmul(
                                    pt_o[ob][:],
                                    attnT[:, jb, :],
                                    v16[:, jb, ob * OW : (ob + 1) * OW],
                                    start=(jb == 0), stop=(jb == JB - 1),
                                )
                        outt = p_sw1.tile([P, O], f32, tag="outt")
                        for ob in range(NOW):
                            nc.vector.tensor_copy(
                                outt[:, ob * OW : (ob + 1) * OW], pt_o[ob][:]
                            )
                        nc.sync.dma_start(out[ibs, :], outt[:])

    nc.compile()
    return nc


_NC_CACHE = {}


def _get_nc(builder, *key):
    k = (builder.__name__,) + key
    if k not in _NC_CACHE:
        _NC_CACHE[k] = builder(*key)
    return _NC_CACHE[k]


def kernel(token_emb, W_q, W_k, W_v, mask=None, _trace=False, _tmpdir=None):
    token_emb = np.asarray(token_emb, np.float32)
    W_q = np.asarray(W_q, np.float32)
    W_k = np.asarray(W_k, np.float32)
    W_v = np.asarray(W_v, np.float32)
    B, S, E = token_emb.shape
    H = W_q.shape[0]
    O = W_v.shape[0]
    SI = S // 2
    EH = E // 2
    HQ = H // 4
    assert 2 * B == N_CORES

    # ---- launch 1: G = W_k^T @ W_q, sharded (e'-half x h-quarter) ----
    nc_g = _get_nc(build_g_nc, E, H)
    wk_h, wk_l = _split16(W_k * 32.0)
    wq_h, wq_l = _split16(W_q * 32.0)
    g_maps = []
    for c in range(N_CORES):
        half, hq = c % 2, c // 2
        hsl = slice(hq * HQ, (hq + 1) * HQ)
        esl = slice(half * EH, (half + 1) * EH)
        g_maps.append(
            {
                "wkh": np.ascontiguousarray(wk_h[hsl]),
                "wkl": np.ascontiguousarray(wk_l[hsl]),
                "wqh": np.ascontiguousarray(wq_h[hsl, esl]),
                "wql": np.ascontiguousarray(wq_l[hsl, esl]),
            }
        )
    res_g = run_bass_kernel_spmd(
        nc_g, g_maps, core_ids=list(range(N_CORES)), trace=_trace,
        tmpdir=(_tmpdir + "/g" if _tmpdir else None),
    )
    G = np.empty((E, E), np.float32)
    for half in range(2):
        esl = slice(half * EH, (half + 1) * EH)
        G[:, esl] = sum(
            res_g.results[2 * q + half]["g_part"].astype(np.float64)
            for q in range(4)
        ).astype(np.float32)
    g_h, g_l = _split16(G)

    # ---- launch 2: attention ----
    nc_main = _get_nc(build_main_nc, S, E, H, O)
    wvt = np.ascontiguousarray(W_v.T).astype(np.float16)
    in_maps = []
    for c in range(N_CORES):
        b, half = divmod(c, 2)
        e = token_emb[b]
        perm = np.concatenate(
            [e[half * SI : (half + 1) * SI], e[(1 - half) * SI : (2 - half) * SI]],
            axis=0,
        )
        et_h, et_l = _split16(perm.T * 32.0)
        in_maps.append(
            {
                "g_h": g_h, "g_l": g_l, "et_h": et_h, "et_l": et_l, "wvt": wvt,
            }
        )
    res = run_bass_kernel_spmd(
        nc_main, in_maps, core_ids=list(range(N_CORES)), trace=_trace,
        tmpdir=(_tmpdir + "/main" if _tmpdir else None),
    )

    out = np.empty((B, S, O), np.float32)
    for c in range(N_CORES):
        b, half = divmod(c, 2)
        out[b, half * SI : (half + 1) * SI] = res.results[c]["out"]
    if _trace:
        kernel._last_results = (res_g, res)
    return out
